# revision 3
# baseline (speedup 1.0000x reference)
"""Trainium2 Bass kernel for varlen (ragged) BERT self-attention.

Strategy: tensor-parallel over heads. 16 heads across 8 NeuronCores ->
2 heads per core. Every core runs an IDENTICAL program (SPMD) on:
  - xt:   full hidden_states, pre-transposed+cast to bf16 on host, (1024, nnz)
  - wt:   this core's slice of Wqkv (q/k/v rows of its 2 heads), as
          matmul-lhsT blocks (8, 128, 384) bf16
  - bias: this core's bias slice (3, 128) f32
Output per core: (nnz, 128) f32 = the 2 owned heads' output columns.
Host concatenates core outputs along axis 1.

On-chip per core:
  1. QKV projection: Y^T[384, nnz] = Wc @ X^T, K=1024 in 8 chunks,
     bias added during PSUM->SBUF eviction (ACT, Copy+bias), cast bf16.
     Gives qT/kT/vT resident in SBUF as [128(=2 heads x 64), nnz].
  2. Per sequence, per head: scoresT[k,q] = (kT rows).T @ qT rows (K=64),
     exp on ACT (scale=1/sqrt(64) folded in), then out^T[65, q] accumulated
     as (v|ones).T @ expT -- the ones column yields the softmax denominator
     for free. PE-transpose + per-partition reciprocal*mul normalizes and
     lands the final (tok, 64) f32 block, DMA'd to DRAM.
No padding, no masking: each sequence is processed at its true length.
"""

import functools
import sys

import numpy as np

for _p in ("/opt/trn_rl_repo",):
    if _p not in sys.path:
        sys.path.append(_p)

import ml_dtypes  # noqa: E402

N_HEADS = 16
HEAD_DIM = 64
DIM = 1024
N_CORES = 8
HEADS_PER_CORE = N_HEADS // N_CORES  # 2


@functools.lru_cache(maxsize=4)
def _build(nnz, lengths):
    """Build + compile the SPMD Bass program for the given ragged lengths."""
    import concourse.mybir as mybir
    import concourse.tile as tile
    from concourse import bacc
    from concourse.masks import make_identity

    f32 = mybir.dt.float32
    bf16 = mybir.dt.bfloat16
    Exp = mybir.ActivationFunctionType.Exp
    Ident = mybir.ActivationFunctionType.Identity

    KC = DIM // 128  # 8 contraction chunks
    M3 = 3 * HEADS_PER_CORE * HEAD_DIM  # 384 output dims per core

    nc = bacc.Bacc("TRN2", target_bir_lowering=False, debug=False)
    xt = nc.declare_dram_parameter("xt", [DIM, nnz], bf16, isOutput=False)
    wt = nc.declare_dram_parameter("wt", [KC, 128, M3], bf16, isOutput=False)
    bias = nc.declare_dram_parameter("bias", [3, 128], f32, isOutput=False)
    out = nc.declare_dram_parameter("out", [nnz, 128], f32, isOutput=True)

    with tile.TileContext(nc) as tc:
        with (
            tc.tile_pool(name="res", bufs=1) as res,
            tc.tile_pool(name="xp", bufs=3) as xp,
            tc.tile_pool(name="esp", bufs=4) as esp,
            tc.tile_pool(name="vgp", bufs=12) as vgp,
            tc.tile_pool(name="osp", bufs=2) as osp,
            tc.tile_pool(name="rsp", bufs=3) as rsp,
            tc.tile_pool(name="obp", bufs=3) as obp,
            tc.tile_pool(name="ps", bufs=2, space="PSUM") as ps,
        ):
            # --- constants / resident tensors ---
            wt_sb = res.tile([128, KC, M3], bf16)
            nc.sync.dma_start(wt_sb[:], wt[:, :, :].rearrange("a p m -> p a m"))
            bias_sb = res.tile([128, 3], f32)
            nc.sync.dma_start(bias_sb[:], bias[:, :].rearrange("a p -> p a"))
            ident_bf = res.tile([128, 128], bf16)
            make_identity(nc, ident_bf[:])
            ident_f32 = res.tile([128, 128], f32)
            make_identity(nc, ident_f32[:])

            qT = res.tile([128, nnz], bf16)
            kT = res.tile([128, nnz], bf16)
            vT = res.tile([128, nnz], bf16)
            qkvT = (qT, kT, vT)

            # --- phase 1: QKV projection ---
            xt_view = xt[:, :].rearrange("(a p) n -> p a n", p=128)
            n_tok_chunks = (nnz + 511) // 512
            for ti in range(n_tok_chunks):
                t0 = ti * 512
                nt = min(512, nnz - t0)
                xt_tile = xp.tile([128, KC, 512], bf16)
                nc.sync.dma_start(
                    xt_tile[:, :, :nt], xt_view[:, :, t0 : t0 + nt]
                )
                for mc in range(3):
                    mm = ps.tile([128, 512], f32, tag="mm")
                    for kc in range(KC):
                        nc.tensor.matmul(
                            mm[:, :nt],
                            wt_sb[:, kc, mc * 128 : (mc + 1) * 128],
                            xt_tile[:, kc, :nt],
                            start=(kc == 0),
                            stop=(kc == KC - 1),
                        )
                    nc.scalar.activation(
                        qkvT[mc][:, t0 : t0 + nt],
                        mm[:, :nt],
                        Ident,
                        bias=bias_sb[:, mc : mc + 1],
                    )

            # --- phase 2: attention per sequence, per head ---
            offset = 0
            for L in lengths:
                if L == 0:
                    continue
                O = offset
                offset += L
                nk = (L + 127) // 128
                nq5 = (L + 511) // 512
                for h in range(HEADS_PER_CORE):
                    p0 = HEAD_DIM * h
                    # v_aug tiles: v natural [ktok, 64] + ones column
                    vags = []
                    for jc in range(nk):
                        nj = min(128, L - jc * 128)
                        vps = ps.tile([128, 64], bf16, tag="tp")
                        nc.tensor.transpose(
                            vps[:nj, :HEAD_DIM],
                            vT[p0 : p0 + HEAD_DIM, O + jc * 128 : O + jc * 128 + nj],
                            ident_bf[p0 : p0 + HEAD_DIM, p0 : p0 + HEAD_DIM],
                        )
                        va = vgp.tile([128, HEAD_DIM + 1], bf16, tag="va")
                        nc.vector.tensor_copy(va[:nj, 0:HEAD_DIM], vps[:nj, :HEAD_DIM])
                        nc.vector.memset(va[:nj, HEAD_DIM : HEAD_DIM + 1], 1.0)
                        vags.append(va)
                    for qc in range(nq5):
                        q0 = qc * 512
                        nq = min(512, L - q0)
                        ov = ps.tile([HEAD_DIM + 1, 512], f32, tag="ov")
                        for jc in range(nk):
                            nj = min(128, L - jc * 128)
                            sps = ps.tile([128, 512], f32, tag="sc")
                            nc.tensor.matmul(
                                sps[:nj, :nq],
                                kT[p0 : p0 + HEAD_DIM, O + jc * 128 : O + jc * 128 + nj],
                                qT[p0 : p0 + HEAD_DIM, O + q0 : O + q0 + nq],
                                start=True,
                                stop=True,
                            )
                            es = esp.tile([128, 512], bf16, tag="es")
                            nc.scalar.activation(
                                es[:nj, :nq], sps[:nj, :nq], Exp, scale=0.125
                            )
                            nc.tensor.matmul(
                                ov[:, :nq],
                                vags[jc][:nj, :],
                                es[:nj, :nq],
                                start=(jc == 0),
                                stop=(jc == nk - 1),
                            )
                        osb = osp.tile([HEAD_DIM + 1, 512], f32, tag="os")
                        nc.vector.tensor_copy(osb[:, :nq], ov[:, :nq])
                        for q1 in range((nq + 127) // 128):
                            r0 = q0 + q1 * 128
                            nqq = min(128, nq - q1 * 128)
                            tps = ps.tile([128, HEAD_DIM + 1], f32, tag="tp")
                            nc.tensor.transpose(
                                tps[:nqq, :],
                                osb[:, q1 * 128 : q1 * 128 + nqq],
                                ident_f32[0 : HEAD_DIM + 1, 0 : HEAD_DIM + 1],
                            )
                            rs = rsp.tile([128, 1], f32, tag="rs")
                            nc.vector.reciprocal(
                                rs[:nqq, :], tps[:nqq, HEAD_DIM : HEAD_DIM + 1]
                            )
                            ob = obp.tile([128, HEAD_DIM], f32, tag="ob")
                            nc.vector.tensor_scalar_mul(
                                ob[:nqq, :], tps[:nqq, 0:HEAD_DIM], rs[:nqq, :]
                            )
                            nc.sync.dma_start(
                                out[O + r0 : O + r0 + nqq, p0 : p0 + HEAD_DIM],
                                ob[:nqq, :],
                            )

    nc.compile()
    return nc


def _prepare(hidden_states, Wqkv_weight, Wqkv_bias, cu_seqlens):
    """Host-side sharding prep. Returns (nc, in_maps)."""
    hs = np.asarray(hidden_states, dtype=np.float32)
    W = np.asarray(Wqkv_weight, dtype=np.float32)
    b = np.asarray(Wqkv_bias, dtype=np.float32).reshape(-1)
    cs = np.asarray(cu_seqlens).astype(np.int64).reshape(-1)
    nnz, dim = hs.shape
    assert dim == DIM and W.shape == (3 * DIM, DIM)
    lengths = tuple(int(cs[i + 1] - cs[i]) for i in range(len(cs) - 1))
    assert sum(lengths) == nnz, (lengths, nnz)

    nc = _build(nnz, lengths)

    xt_np = np.ascontiguousarray(hs.T).astype(ml_dtypes.bfloat16)
    in_maps = []
    for c in range(N_CORES):
        r0 = c * HEADS_PER_CORE * HEAD_DIM  # 128c
        rows = []
        biases = []
        for part in range(3):  # q, k, v
            rows.append(W[part * DIM + r0 : part * DIM + r0 + 128, :])
            biases.append(b[part * DIM + r0 : part * DIM + r0 + 128])
        Wc = np.concatenate(rows, axis=0)  # (384, 1024)
        wt_np = np.ascontiguousarray(Wc.T.reshape(DIM // 128, 128, 384)).astype(
            ml_dtypes.bfloat16
        )
        bias_np = np.ascontiguousarray(np.stack(biases, axis=0))  # (3, 128)
        in_maps.append({"xt": xt_np, "wt": wt_np, "bias": bias_np})
    return nc, in_maps


def kernel(hidden_states, Wqkv_weight, Wqkv_bias, cu_seqlens, max_seqlen=None):
    from concourse.bass_utils import run_bass_kernel_spmd

    nc, in_maps = _prepare(hidden_states, Wqkv_weight, Wqkv_bias, cu_seqlens)
    res = run_bass_kernel_spmd(nc, in_maps, list(range(N_CORES)))
    out = np.concatenate(
        [res.results[c]["out"] for c in range(N_CORES)], axis=1
    )
    return np.ascontiguousarray(out, dtype=np.float32)


# revision 5
# speedup vs baseline: 1.1651x; 1.1651x over previous
"""Trainium2 Bass kernel for varlen (ragged) BERT self-attention.

Strategy: tensor-parallel over heads. 16 heads across 8 NeuronCores ->
2 heads per core. Every core runs an IDENTICAL program (SPMD) on:
  - xt:   full hidden_states, pre-transposed+cast to bf16 on host, (1024, nnz)
  - wt:   this core's slice of Wqkv (q/k/v rows of its 2 heads), as
          matmul-lhsT blocks (8, 128, 384) bf16
  - bias: this core's bias slice (3, 128) f32
Output per core: (nnz, 128) f32 = the 2 owned heads' output columns.
Host concatenates core outputs along axis 1.

On-chip per core:
  1. QKV projection: Y^T[384, nnz] = Wc @ X^T, K=1024 in 8 chunks,
     bias added during PSUM->SBUF eviction (ACT, Identity+bias), cast bf16.
     Gives qT/kT/vT resident in SBUF as [128(=2 heads x 64), nnz].
  2. Per sequence, per head: scoresT[k,q] = (kT rows).T @ qT rows (K=64),
     exp on ACT (scale=1/sqrt(64) folded in), then out^T[65, q] accumulated
     as (v|ones).T @ expT -- the ones column yields the softmax denominator
     for free. PE-transpose + per-partition reciprocal*mul normalizes and
     lands the final (tok, 64) f32 block, DMA'd to DRAM.

Emission order interleaves per-sequence attention into the QKV chunk
stream (chunks processed back-to-front, sequences become ready
largest-first) so the PE instruction stream stays dense end-to-end --
otherwise the HAM clock gate re-throttles the tensor engine to half
clock during the attention phase.

No padding, no masking: each sequence is processed at its true length.
"""

import functools
import sys

import numpy as np

for _p in ("/opt/trn_rl_repo",):
    if _p not in sys.path:
        sys.path.append(_p)

import ml_dtypes  # noqa: E402

N_HEADS = 16
HEAD_DIM = 64
DIM = 1024
N_CORES = 8
HEADS_PER_CORE = N_HEADS // N_CORES  # 2


@functools.lru_cache(maxsize=4)
def _build(nnz, lengths):
    """Build + compile the SPMD Bass program for the given ragged lengths."""
    import concourse.mybir as mybir
    import concourse.tile as tile
    from concourse import bacc
    from concourse.masks import make_identity

    f32 = mybir.dt.float32
    bf16 = mybir.dt.bfloat16
    Exp = mybir.ActivationFunctionType.Exp
    Ident = mybir.ActivationFunctionType.Identity

    KC = DIM // 128  # 8 contraction chunks
    M3 = 3 * HEADS_PER_CORE * HEAD_DIM  # 384 output dims per core
    D = HEAD_DIM

    nc = bacc.Bacc("TRN2", target_bir_lowering=False, debug=False)
    xt = nc.declare_dram_parameter("xt", [DIM, nnz], bf16, isOutput=False)
    wt = nc.declare_dram_parameter("wt", [KC, 128, M3], bf16, isOutput=False)
    bias = nc.declare_dram_parameter("bias", [3, 128], f32, isOutput=False)
    out = nc.declare_dram_parameter("out", [nnz, 128], f32, isOutput=True)

    # sequence table: (offset, length), emitted when their token range is
    # covered by the emitted qkv chunks (chunks go back-to-front)
    seqs = []
    off = 0
    for L in lengths:
        if L > 0:
            seqs.append((off, L))
        off += L
    n_tok_chunks = (nnz + 511) // 512

    with tile.TileContext(nc) as tc:
        with (
            tc.tile_pool(name="res", bufs=1) as res,
            tc.tile_pool(name="xp", bufs=4) as xp,
            tc.tile_pool(name="esp", bufs=6) as esp,
            tc.tile_pool(name="vgp", bufs=20) as vgp,
            tc.tile_pool(name="osp", bufs=3) as osp,
            tc.tile_pool(name="rsp", bufs=3) as rsp,
            tc.tile_pool(name="obp", bufs=4) as obp,
            tc.tile_pool(name="ps", bufs=2, space="PSUM") as ps,
        ):
            # --- constants / resident tensors ---
            wt_sb = res.tile([128, KC, M3], bf16)
            nc.sync.dma_start(wt_sb[:], wt[:, :, :].rearrange("a p m -> p a m"))
            bias_sb = res.tile([128, 3], f32)
            nc.sync.dma_start(bias_sb[:], bias[:, :].rearrange("a p -> p a"))
            ident_bf = res.tile([128, 128], bf16)
            make_identity(nc, ident_bf[:])
            ident_f32 = res.tile([128, 128], f32)
            make_identity(nc, ident_f32[:])

            qT = res.tile([128, nnz], bf16)
            kT = res.tile([128, nnz], bf16)
            vT = res.tile([128, nnz], bf16)
            qkvT = (qT, kT, vT)

            xt_view = xt[:, :].rearrange("(a p) n -> p a n", p=128)

            def emit_qkv_chunk(ti):
                t0 = ti * 512
                nt = min(512, nnz - t0)
                xt_tile = xp.tile([128, KC, 512], bf16, tag="xt")
                nc.sync.dma_start(xt_tile[:, :, :nt], xt_view[:, :, t0 : t0 + nt])
                for mc in range(3):
                    mm = ps.tile([128, 512], f32, tag="mm", bufs=2)
                    for kc in range(KC):
                        nc.tensor.matmul(
                            mm[:, :nt],
                            wt_sb[:, kc, mc * 128 : (mc + 1) * 128],
                            xt_tile[:, kc, :nt],
                            start=(kc == 0),
                            stop=(kc == KC - 1),
                        )
                    nc.scalar.activation(
                        qkvT[mc][:, t0 : t0 + nt],
                        mm[:, :nt],
                        Ident,
                        bias=bias_sb[:, mc : mc + 1],
                    )

            def emit_attention(O, L):
                nk = (L + 127) // 128
                nq5 = (L + 511) // 512
                HP = HEADS_PER_CORE
                # v_aug tiles (v natural + ones column), both heads
                vags = {}
                for h in range(HP):
                    p0 = D * h
                    for jc in range(nk):
                        nj = min(128, L - jc * 128)
                        vps = ps.tile([128, D], bf16, tag="tp", bufs=1)
                        nc.tensor.transpose(
                            vps[:nj, :D],
                            vT[p0 : p0 + D, O + jc * 128 : O + jc * 128 + nj],
                            ident_bf[p0 : p0 + D, p0 : p0 + D],
                        )
                        va = vgp.tile([128, D + 1], bf16, tag="va")
                        nc.vector.tensor_copy(va[:nj, 0:D], vps[:nj, :D])
                        nc.vector.memset(va[:nj, D : D + 1], 1.0)
                        vags[(h, jc)] = va
                for qc in range(nq5):
                    q0 = qc * 512
                    nq = min(512, L - q0)
                    ovs = [
                        ps.tile([D + 1, 512], f32, tag="ov", bufs=2, name=f"ov{h}")
                        for h in range(HP)
                    ]
                    for jc in range(nk):
                        nj = min(128, L - jc * 128)
                        for h in range(HP):
                            p0 = D * h
                            sps = ps.tile([128, 512], f32, tag="sc", bufs=3)
                            nc.tensor.matmul(
                                sps[:nj, :nq],
                                kT[p0 : p0 + D, O + jc * 128 : O + jc * 128 + nj],
                                qT[p0 : p0 + D, O + q0 : O + q0 + nq],
                                start=True,
                                stop=True,
                            )
                            es = esp.tile([128, 512], bf16, tag="es")
                            nc.scalar.activation(
                                es[:nj, :nq], sps[:nj, :nq], Exp, scale=0.125
                            )
                            nc.tensor.matmul(
                                ovs[h][:, :nq],
                                vags[(h, jc)][:nj, :],
                                es[:nj, :nq],
                                start=(jc == 0),
                                stop=(jc == nk - 1),
                            )
                    for h in range(HP):
                        p0 = D * h
                        osb = osp.tile([D + 1, 512], f32, tag="os")
                        nc.vector.tensor_copy(osb[:, :nq], ovs[h][:, :nq])
                        for q1 in range((nq + 127) // 128):
                            r0 = q0 + q1 * 128
                            nqq = min(128, nq - q1 * 128)
                            tps = ps.tile([128, D + 1], f32, tag="tp", bufs=1)
                            nc.tensor.transpose(
                                tps[:nqq, :],
                                osb[:, q1 * 128 : q1 * 128 + nqq],
                                ident_f32[0 : D + 1, 0 : D + 1],
                            )
                            rs = rsp.tile([128, 1], f32, tag="rs")
                            nc.vector.reciprocal(rs[:nqq, :], tps[:nqq, D : D + 1])
                            ob = obp.tile([128, D], f32, tag="ob")
                            nc.vector.tensor_scalar_mul(
                                ob[:nqq, :], tps[:nqq, 0:D], rs[:nqq, :]
                            )
                            nc.sync.dma_start(
                                out[O + r0 : O + r0 + nqq, p0 : p0 + D],
                                ob[:nqq, :],
                            )

            # --- interleaved emission ---
            # chunks back-to-front; a sequence is ready once all chunks
            # covering [O, O+L) are emitted, i.e. O >= 512*ti
            pending = sorted(seqs, key=lambda s: s[0], reverse=True)
            pi = 0
            for ti in range(n_tok_chunks - 1, -1, -1):
                emit_qkv_chunk(ti)
                while pi < len(pending) and pending[pi][0] >= 512 * ti:
                    emit_attention(*pending[pi])
                    pi += 1
            while pi < len(pending):
                emit_attention(*pending[pi])
                pi += 1

    nc.compile()
    return nc


def _prepare(hidden_states, Wqkv_weight, Wqkv_bias, cu_seqlens):
    """Host-side sharding prep. Returns (nc, in_maps)."""
    hs = np.asarray(hidden_states, dtype=np.float32)
    W = np.asarray(Wqkv_weight, dtype=np.float32)
    b = np.asarray(Wqkv_bias, dtype=np.float32).reshape(-1)
    cs = np.asarray(cu_seqlens).astype(np.int64).reshape(-1)
    nnz, dim = hs.shape
    assert dim == DIM and W.shape == (3 * DIM, DIM)
    lengths = tuple(int(cs[i + 1] - cs[i]) for i in range(len(cs) - 1))
    assert sum(lengths) == nnz, (lengths, nnz)

    nc = _build(nnz, lengths)

    xt_np = np.ascontiguousarray(hs.T).astype(ml_dtypes.bfloat16)
    in_maps = []
    for c in range(N_CORES):
        r0 = c * HEADS_PER_CORE * HEAD_DIM  # 128c
        rows = []
        biases = []
        for part in range(3):  # q, k, v
            rows.append(W[part * DIM + r0 : part * DIM + r0 + 128, :])
            biases.append(b[part * DIM + r0 : part * DIM + r0 + 128])
        Wc = np.concatenate(rows, axis=0)  # (384, 1024)
        wt_np = np.ascontiguousarray(Wc.T.reshape(DIM // 128, 128, 384)).astype(
            ml_dtypes.bfloat16
        )
        bias_np = np.ascontiguousarray(np.stack(biases, axis=0))  # (3, 128)
        in_maps.append({"xt": xt_np, "wt": wt_np, "bias": bias_np})
    return nc, in_maps


def kernel(hidden_states, Wqkv_weight, Wqkv_bias, cu_seqlens, max_seqlen=None):
    from concourse.bass_utils import run_bass_kernel_spmd

    nc, in_maps = _prepare(hidden_states, Wqkv_weight, Wqkv_bias, cu_seqlens)
    res = run_bass_kernel_spmd(nc, in_maps, list(range(N_CORES)))
    out = np.concatenate(
        [res.results[c]["out"] for c in range(N_CORES)], axis=1
    )
    return np.ascontiguousarray(out, dtype=np.float32)
